# revision 37
# baseline (speedup 1.0000x reference)
"""GCGRUCell (SplineConv-based GRU cell) Trainium2 kernel.

Strategy (8 NeuronCores, SPMD):
- Nodes are partitioned contiguously across cores (750/core, padded to 768);
  each core owns all edges whose destination lies in its node range
  (host groups edges by destination; source features are replicated so no
  halo exchange is needed).
- The six SplineConvs share one sparse structure: for each edge, the
  degree-1 B-spline basis over K=5^3 kernels factorizes as
  kron(v1, v2, v3) with v_d[j] = relu(1 - |j - u_d|) (5-wide hat weights).
  Per destination node n we form T_n[k, i] = sum_{e->n} basis[e,k] *
  xh[src_e, i] with xh = [x | hidden] (96 features), via one PE matmul per
  node: lhsT = A (64 edge rows x 128 basis cols, built on-device with two
  kron tensor_tensor ops), rhs = gathered source features (dma_gather).
- All six convs then reduce to dense matmuls over T: out = sum_i
  T[:, i, :]^T @ W[., i, .] accumulated in PSUM, plus a root/bias matmul
  (contraction rows = raw x/h features + a ones row). 1/deg mean
  normalization is folded into v1 on the host.
- GRU combine (sigmoid/tanh/elementwise) runs on ACT+DVE per node block.
"""

import os
import numpy as np

# Device-side dma_gather works but its Q7 descriptor generation runs at
# ~9 ns/row (437 us busy for the full gather) and dominates the kernel, so
# the gather reorder is done host-side and streamed as one contiguous DMA.
USE_DMA_GATHER = False  # Q7 desc-gen ~9ns/row made this a net loss
USE_LDW_OPT = os.environ.get("KERN_LDWOPT", "") != ""
# Build the basis matrix A on host (numpy kron) instead of on DVE: DVE is a
# critical engine (PSUM->SBUF copies) while DMA has headroom.
HOST_A = os.environ.get("KERN_DEV_A", "") == ""

# ---------------- problem constants (hardcoded per contract) ----------------
N, E, CIN, CHID, DIM, KS = 6000, 192000, 32, 64, 3, 5
K = KS ** DIM
NCORES = 8
NPN = N // NCORES          # 750 real nodes per core
BN = 128                   # nodes per block
NBLK = 6                   # blocks per core (768 padded nodes)
NPC = BN * NBLK            # 768 padded nodes per core
FEAT = CIN + CHID          # 96
ELEM = 128                 # gather row length (fp16): 96 feats + 32 pad
RPN = 64                   # edge rows per node (max supported degree)
ROWS_BLK = BN * RPN        # 8192
CH_BLK = ROWS_BLK // 128   # 64 chunks per block
OC = 3 * CHID              # 192 x-conv output columns (r|z|n)
OH = 2 * CHID              # 128 h-conv output columns (r|z)

_cache = {}


def _patch_tile_drain(tile_mod, mybir):
    """This walrus build rejects >1-2 sync waits on the tail Drain
    ("Too many sync wait commands"); spread waits across nops instead."""
    if getattr(tile_mod.TileContext, "_drain_patched", False):
        return

    def _drain_and_barrier(self, tick_clock, wait_clock):
        drain_inst = self.nc.sync.drain()
        wait_clock.add_sem_waits(
            drain_inst.ins, tile_mod.ScopedClock({None: tick_clock.global_clock})
        )
        si = drain_inst.ins.sync_info
        waits = list(si.on_wait or [])
        if len(waits) > 1:
            si.on_wait = [waits[0]]
            for w in waits[1:]:
                nop = self.nc.sync.nop(nofuse=True)
                nsi = nop.ins.sync_info
                if nsi is None:
                    nop.ins.sync_info = mybir.SyncInfo(on_wait=[w], on_update=[])
                else:
                    nsi.on_wait = [w]
        self.nc.all_engine_barrier()
        assert self.sems is not None
        popped = self.nc._tile_sem_poison_stack.pop()
        assert popped is self._sem_poison
        self.nc.clear_and_free_semaphores(list(self.sems.allocated().values()))
        self.nc.all_engine_barrier()

    tile_mod.TileContext._drain_and_barrier = _drain_and_barrier
    tile_mod.TileContext._drain_patched = True


def _patch_ldw_opt():
    """compile_bir_kernel hardcodes --enable-ldw-opt=false; rewrite it."""
    from concourse import bass_utils

    if getattr(bass_utils, "_ldw_patched", False):
        return
    orig = bass_utils.run_command

    def run_command(cmd, *a, **kw):
        cmd = [c.replace("--enable-ldw-opt=false", "--enable-ldw-opt=true")
               if isinstance(c, str) else c for c in cmd]
        return orig(cmd, *a, **kw)

    bass_utils.run_command = run_command
    bass_utils._ldw_patched = True


def _build_program():
    import dataclasses
    import concourse.bass as bass
    import concourse.tile as tile
    from concourse import mybir, bacc, library_config

    _patch_tile_drain(tile, mybir)
    if USE_LDW_OPT:
        _patch_ldw_opt()

    f16, f32, i16 = mybir.dt.float16, mybir.dt.float32, mybir.dt.int16
    alu = mybir.AluOpType
    act = mybir.ActivationFunctionType
    GE = 128 if USE_DMA_GATHER else FEAT  # gathered row length

    nc = bacc.Bacc()
    if USE_DMA_GATHER:
        d_xh = nc.declare_dram_parameter("xh", [N, GE], f16, isOutput=False)
        d_idx = nc.declare_dram_parameter("idx", [128, NBLK * 512], i16, isOutput=False)
    else:
        d_gf = nc.declare_dram_parameter(
            "gf", [128, NBLK * CH_BLK * GE], f16, isOutput=False
        )
    if HOST_A:
        d_am = nc.declare_dram_parameter(
            "am", [128, NBLK * CH_BLK * 128], f16, isOutput=False
        )
    else:
        d_vv = nc.declare_dram_parameter(
            "vv", [128, NBLK * CH_BLK * 15], f16, isOutput=False
        )
    d_gz = nc.declare_dram_parameter("gz", [64, CH_BLK * FEAT], f16, isOutput=False)
    d_wx = nc.declare_dram_parameter("wx", [128, CIN * OC], f16, isOutput=False)
    d_wh = nc.declare_dram_parameter("wh", [128, CHID * OH], f16, isOutput=False)
    d_xht = nc.declare_dram_parameter("xht", [FEAT + 1, NPC], f16, isOutput=False)
    d_rx = nc.declare_dram_parameter("rx", [FEAT + 1, OC], f16, isOutput=False)
    d_rh = nc.declare_dram_parameter("rh", [FEAT + 1, OH], f16, isOutput=False)
    d_hid = nc.declare_dram_parameter("hid", [128, NBLK * CHID], f32, isOutput=False)
    d_out = nc.declare_dram_parameter("out", [128, NBLK * CHID], f32, isOutput=True)

    def rep(ap, pattern, extra_off=0):
        return dataclasses.replace(ap, ap=pattern, offset=ap.offset + extra_off)

    with tile.TileContext(nc) as tc:
        with (
            tc.tile_pool(name="const", bufs=1) as cp,
            tc.tile_pool(name="apool", bufs=2) as apool,
            tc.tile_pool(name="gp", bufs=2) as gp,
            tc.tile_pool(name="tp", bufs=2) as tp,
            tc.tile_pool(name="sp", bufs=2) as sp,
            tc.tile_pool(name="psc", bufs=4, space="PSUM") as psc,
            tc.tile_pool(name="ppx", bufs=2, space="PSUM") as ppx,
            tc.tile_pool(name="pph", bufs=2, space="PSUM") as pph,
        ):
            wx_sb = cp.tile([128, CIN, OC], f16)
            wh_sb = cp.tile([128, CHID, OH], f16)
            xht_sb = cp.tile([FEAT + 1, NPC], f16)
            rx_sb = cp.tile([FEAT + 1, OC], f16)
            rh_sb = cp.tile([FEAT + 1, OH], f16)
            hid_sb = cp.tile([128, NBLK, CHID], f32)

            gfv = d_gf[:].rearrange("p (b c e) -> p b c e", c=CH_BLK, e=GE)
            amv = d_am[:].rearrange("p (b c e) -> p b c e", c=CH_BLK, e=128)

            # Persistent double-buffered block-diagonal feature tiles, laid
            # out [128, half, CH_BLK, 96] so each DMA half is one contiguous
            # 12KB/partition run (full-rate descriptors; the old
            # [128, c, 192] interleave paid the sub-512B 2x penalty).
            # Rows 0:64 hold node A's features in half 0, rows 64:128 node
            # B's in half 1; the opposite halves are zeroed once.
            gtiles = [cp.tile([128, 2, CH_BLK, GE], f16, name=f"gblk{i}",
                              tag=f"gblk{i}") for i in range(2)]
            gzv = d_gz[:].rearrange("p (c e) -> p c e", e=GE)

            def emit_gf_dma(b):
                """Prefetch block b's gathered features + basis matrix.
                The basis matrix comes in two halves so the first chunks'
                ldweights can start after ~3us instead of ~6us."""
                g = gtiles[b % 2]
                nc.sync.dma_start(g[0:64, 0], gfv[0:64, b])
                nc.sync.dma_start(g[64:128, 1], gfv[64:128, b])
                a_t = apool.tile([128, CH_BLK, 128], f16)
                h = CH_BLK // 2
                nc.sync.dma_start(a_t[:, 0:h, :], amv[:, b, 0:h])
                nc.sync.dma_start(a_t[:, h:CH_BLK, :], amv[:, b, h:CH_BLK])
                return g, a_t

            # Block-diagonal zero halves, spread across otherwise-idle
            # engines so gtile0 is ready before block 0's first copies need
            # DVE (~10.5us) and gtile1 before block 1 (~17us). A zero-DMA
            # would serialize ahead of the critical gf0/am0 transfers.
            nc.vector.memset(gtiles[0][64:128, 0], 0)   # DVE  ~6.5us
            nc.gpsimd.memset(gtiles[0][0:64, 1], 0)     # Pool ~5.2us
            nc.gpsimd.memset(gtiles[1][64:128, 0], 0)   # Pool ~5.2us
            nc.gpsimd.memset(gtiles[1][0:64, 1], 0)     # Pool ~5.2us

            # SP HWDGE queue order = DMA priority order: blocks 0 and 1
            # first (compute can start ~9us in), then the back-part consts
            # (not needed until back(0) interleaves into front(1)), then the
            # steady-state prefetch stream (emitted inside the block loop).
            pend = [emit_gf_dma(0), emit_gf_dma(1)]
            nc.sync.dma_start(wx_sb[:], d_wx[:].rearrange("p (i o) -> p i o", o=OC))
            nc.sync.dma_start(wh_sb[:], d_wh[:].rearrange("p (i o) -> p i o", o=OH))
            nc.sync.dma_start(xht_sb[:], d_xht[:])
            nc.sync.dma_start(rx_sb[:], d_rx[:])
            nc.sync.dma_start(rh_sb[:], d_rh[:])
            nc.sync.dma_start(hid_sb[:], d_hid[:].rearrange("p (b c) -> p b c", c=CHID))

            def emit_front_chunks(b, g, a_t, t_t):
                """Scatter matmuls + batched T copies for block b.

                Each 128-row chunk is two nodes x 64 edge rows; the two are
                computed by two row-tiled matmuls (contraction rows 0:64 and
                64:128 via base_partition, concurrent sub-array groups)
                writing disjoint 96-col ranges of the same PSUM bank. Two
                chunks share one PSUM tile so each drain copy moves 384
                elements (amortizes the ~125-143ns PSUM-access overhead).

                Yields after each chunk-pair so back-part matmuls of the
                previous block can interleave into the PE queue and fill
                the copy-drain gaps.
                """
                for c in range(0, CH_BLK, 2):
                    ps = psc.tile([128, 4 * FEAT], f32)
                    for j in range(2):
                        cc = c + j
                        o = 2 * j * FEAT
                        nc.tensor.matmul(
                            ps[:, o:o + 2 * FEAT], a_t[:, cc, :], g[:, :, cc, :],
                            start=True, stop=True,
                        )
                    if (c // 2) % 2 == 0:
                        nc.vector.tensor_copy(t_t[:, 2 * c:2 * c + 4, :], ps[:])
                    else:
                        nc.scalar.copy(t_t[:, 2 * c:2 * c + 4, :], ps[:])
                    yield
                return

            # per-block conv outputs stashed here; the GRU combine runs ONCE,
            # batched over all blocks, after the pipeline drains. Keeping the
            # GRU chain out of the per-block DVE/ACT streams matters: its
            # ops become dependency-ready just before the next block's first
            # T copies, so the (readiness-ordered) scheduler would slot the
            # whole serial chain ahead of them and stall PE on PSUM drain.
            sxp = cp.tile([128, NBLK, OC], f32)
            sxh = cp.tile([128, NBLK, OH], f32)

            def back_mm_thunks(b, t_t):
                """The dense conv matmuls for block b as a list of thunks,
                interleaved into the next block's front emission."""
                t_iv = t_t[:].rearrange("p n i -> p i n")
                px = ppx.tile([128, OC], f32)
                ph = pph.tile([128, OH], f32)
                thunks = []
                for i in range(CIN):
                    thunks.append(lambda i=i: nc.tensor.matmul(
                        px[:], t_iv[:, i, :], wx_sb[:, i, :],
                        start=(i == 0), stop=False,
                    ))
                for i in range(CHID):
                    thunks.append(lambda i=i: nc.tensor.matmul(
                        ph[:], t_iv[:, CIN + i, :], wh_sb[:, i, :],
                        start=(i == 0), stop=False,
                    ))
                xht_blk = xht_sb[:, b * BN:(b + 1) * BN]
                thunks.append(lambda: nc.tensor.matmul(
                    px[:], xht_blk, rx_sb[:], start=False, stop=True))
                thunks.append(lambda: nc.tensor.matmul(
                    ph[:], xht_blk, rh_sb[:], start=False, stop=True))
                thunks.append(lambda: nc.scalar.copy(sxp[:, b, :], px[:]))
                thunks.append(lambda: nc.scalar.copy(sxh[:, b, :], ph[:]))
                return thunks, None

            def emit_batched_gru(lo, hi):
                """GRU combine for blocks [lo, hi) + one output DMA.

                r and z share one sigmoid: the stash layout is
                [px_r|px_z|px_n] / [ph_r|ph_z], so a12 = px[0:128]+ph and
                rz = sigmoid(a12) computes both gates per block.
                """
                C = CHID
                nb = hi - lo
                a12 = sp.tile([128, nb, 2 * C], f32, tag=f"a12_{lo}")
                nc.vector.tensor_tensor(out=a12[:], in0=sxp[:, lo:hi, 0:2 * C],
                                        in1=sxh[:, lo:hi, :], op=alu.add)
                rz = sp.tile([128, nb, 2 * C], f32, tag=f"rz_{lo}")
                nc.scalar.activation(rz[:], a12[:], act.Sigmoid)
                t1 = sp.tile([128, nb, C], f32, tag=f"t1_{lo}")
                nc.vector.tensor_tensor(out=t1[:], in0=rz[:, :, 0:C],
                                        in1=sxh[:, lo:hi, 0:C], op=alu.mult)
                t2 = sp.tile([128, nb, C], f32, tag=f"t2_{lo}")
                nc.vector.tensor_tensor(out=t2[:], in0=sxp[:, lo:hi, 2 * C:3 * C],
                                        in1=t1[:], op=alu.add)
                nn_ = sp.tile([128, nb, C], f32, tag=f"nn_{lo}")
                nc.scalar.activation(nn_[:], t2[:], act.Tanh)
                t3 = sp.tile([128, nb, C], f32, tag=f"t3_{lo}")
                nc.vector.tensor_tensor(out=t3[:], in0=hid_sb[:, lo:hi, :], in1=nn_[:],
                                        op=alu.subtract)
                t4 = sp.tile([128, nb, C], f32, tag=f"t4_{lo}")
                nc.vector.tensor_tensor(out=t4[:], in0=rz[:, :, C:2 * C],
                                        in1=t3[:], op=alu.mult)
                hn = sp.tile([128, nb, C], f32, tag=f"hn_{lo}")
                nc.vector.tensor_tensor(out=hn[:], in0=nn_[:], in1=t4[:], op=alu.add)
                nc.scalar.dma_start(
                    d_out[:].rearrange("p (b c) -> p b c", c=CHID)[:, lo:hi, :],
                    hn[:],
                )

            # Software pipeline, interleaved at chunk granularity: while
            # front(b) crawls at copy-drain speed, back(b-1) matmuls slot
            # into the PE queue between front chunk-pairs.
            pending_back = None   # (b-1, thunks, pb)
            for b in range(NBLK):
                g, a_t = pend.pop(0)
                t_t = tp.tile([128, BN, FEAT], f16)
                for _ in emit_front_chunks(b, g, a_t, t_t):
                    if pending_back is not None:
                        thunks = pending_back[1]
                        for _ in range(4):
                            if thunks:
                                thunks.pop(0)()
                if b + 2 < NBLK:
                    pend.append(emit_gf_dma(b + 2))
                if pending_back is not None:
                    bb, thunks, pbt = pending_back
                    for th in thunks:
                        th()
                    if bb == 3:
                        # GRU for blocks 0-3 overlaps blocks 4-5's compute
                        emit_batched_gru(0, 4)
                    elif bb == 4:
                        # block 4's combine overlaps block 5's back part;
                        # only block 5's small combine remains in the tail
                        emit_batched_gru(4, 5)
                pending_back = (b,) + back_mm_thunks(b, t_t)
            # drain the pipeline tail: last back part, then the rest of GRU
            bb, thunks, pbt = pending_back
            for th in thunks:
                th()
            emit_batched_gru(NBLK - 1, NBLK)

    nc.compile()
    return nc


def _plan_inputs(x, hidden, edge_index, edge_attr,
                 W_xr, root_xr, b_xr, W_hr, root_hr, b_hr,
                 W_xz, root_xz, b_xz, W_hz, root_hz, b_hz,
                 W_xn, root_xn, b_xn, W_hn=None, root_hn=None, b_hn=None):
    """Host-side sharding: group edges by destination core/node, build the
    per-core swizzled index/basis arrays and packed weights."""
    src = np.asarray(edge_index[0], np.int64)
    dst = np.asarray(edge_index[1], np.int64)
    x = np.asarray(x, np.float32)
    hidden = np.asarray(hidden, np.float32)
    edge_attr = np.asarray(edge_attr, np.float32)
    GE = 128 if USE_DMA_GATHER else FEAT

    deg = np.bincount(dst, minlength=N)
    if deg.max() > RPN:
        raise NotImplementedError(f"max degree {deg.max()} exceeds {RPN}")
    recip = 1.0 / np.maximum(deg, 1).astype(np.float32)

    # hat-basis weights per edge/dim: v[e, d, j] = relu(1 - |j - u_d|)
    u = edge_attr * (KS - 1)
    jj = np.arange(KS, dtype=np.float32)
    v = np.maximum(0.0, 1.0 - np.abs(jj[None, None, :] - u[:, :, None]))
    v1s = v[:, 0, :] * recip[dst][:, None]

    xh = np.zeros((N, GE), np.float16)
    xh[:, 0:CIN] = x
    xh[:, CIN:FEAT] = hidden
    wx = np.zeros((128, CIN, OC), np.float16)
    wx[:K, :, 0:CHID] = W_xr
    wx[:K, :, CHID:2 * CHID] = W_xz
    wx[:K, :, 2 * CHID:] = W_xn
    wh = np.zeros((128, CHID, OH), np.float16)
    wh[:K, :, 0:CHID] = W_hr
    wh[:K, :, CHID:] = W_hz
    rx = np.zeros((FEAT + 1, OC), np.float16)
    rx[0:CIN, 0:CHID] = root_xr
    rx[0:CIN, CHID:2 * CHID] = root_xz
    rx[0:CIN, 2 * CHID:] = root_xn
    rx[FEAT, :] = np.concatenate([b_xr, b_xz, b_xn]).astype(np.float16)
    rh = np.zeros((FEAT + 1, OH), np.float16)
    rh[CIN:FEAT, 0:CHID] = root_hr
    rh[CIN:FEAT, CHID:] = root_hz
    rh[FEAT, :] = np.concatenate([b_hr, b_hz]).astype(np.float16)

    in_maps = []
    for c in range(NCORES):
        lo = c * NPN
        sel = np.nonzero((dst >= lo) & (dst < lo + NPN))[0]
        order = sel[np.argsort(dst[sel], kind="stable")]
        sdst = dst[order]
        first = np.searchsorted(sdst, sdst, side="left")
        pos = np.arange(len(order)) - first
        rows = (sdst - lo) * RPN + pos  # in [0, NPC*RPN)

        esrc = np.zeros(NPC * RPN, np.int32)
        esrc[rows] = src[order]

        m = {
            "wx": np.ascontiguousarray(wx.reshape(128, CIN * OC)),
            "wh": np.ascontiguousarray(wh.reshape(128, CHID * OH)),
            "rx": rx, "rh": rh,
            "gz": np.zeros((64, CH_BLK * FEAT), np.float16),
        }

        if HOST_A:
            # A row = kron(v1*recip, v2, v3), fp32 accumulate, one fp16 round
            a_full = np.zeros((NPC * RPN, 128), np.float16)
            t25 = (v1s[order][:, :, None] * v[order][:, 1, None, :]).reshape(-1, 25)
            a_full[rows, 0:K] = (
                t25[:, :, None] * v[order][:, 2, None, :]
            ).reshape(-1, K)
            m["am"] = np.ascontiguousarray(
                a_full.reshape(NBLK * CH_BLK, 128, 128).transpose(1, 0, 2)
            ).reshape(128, NBLK * CH_BLK * 128)
        else:
            vvr = np.zeros((NPC * RPN, 15), np.float16)
            vvr[rows, 0:5] = v1s[order]
            vvr[rows, 5:10] = v[order][:, 1, :]
            vvr[rows, 10:15] = v[order][:, 2, :]
            m["vv"] = np.ascontiguousarray(
                vvr.reshape(NBLK * CH_BLK, 128, 15).transpose(1, 0, 2)
            ).reshape(128, NBLK * CH_BLK * 15)

        if USE_DMA_GATHER:
            m["xh"] = xh
            idxw = np.zeros((128, NBLK * 512), np.int16)
            eb = esrc.astype(np.int16).reshape(NBLK, ROWS_BLK)
            for b in range(NBLK):
                w = eb[b].reshape(512, 16).T
                idxw[:, b * 512:(b + 1) * 512] = np.tile(w, (8, 1))
            m["idx"] = idxw
        else:
            gath = xh[esrc]  # [NPC*RPN, GE]
            gf = gath.reshape(NBLK, CH_BLK, 128, GE).transpose(2, 0, 1, 3)
            m["gf"] = np.ascontiguousarray(gf).reshape(128, NBLK * CH_BLK * GE)

        xht = np.zeros((FEAT + 1, NPC), np.float16)
        xht[0:CIN, 0:NPN] = x[lo:lo + NPN].T
        xht[CIN:FEAT, 0:NPN] = hidden[lo:lo + NPN].T
        xht[FEAT, :] = 1.0
        m["xht"] = xht

        hid_pad = np.zeros((NPC, CHID), np.float32)
        hid_pad[0:NPN] = hidden[lo:lo + NPN]
        m["hid"] = np.ascontiguousarray(
            hid_pad.reshape(NBLK, 128, CHID).transpose(1, 0, 2)
        ).reshape(128, NBLK * CHID)

        in_maps.append(m)
    return in_maps


def kernel(**inputs):
    from concourse.bass_utils import run_bass_kernel_spmd

    if "nc" not in _cache:
        _cache["nc"] = _build_program()
    nc = _cache["nc"]

    in_maps = _plan_inputs(**inputs)
    res = run_bass_kernel_spmd(nc, in_maps, list(range(NCORES)))
    _cache["last_res"] = res

    out = np.empty((N, CHID), np.float32)
    for c in range(NCORES):
        blk = res.results[c]["out"].reshape(128, NBLK, CHID)
        full = blk.transpose(1, 0, 2).reshape(NPC, CHID)
        out[c * NPN:(c + 1) * NPN] = full[:NPN]
    return out

